# revision 1
# baseline (speedup 1.0000x reference)
"""Trainium2 Bass kernel for nn_AttentionMemoryEntry (moe_routing).

Strategy:
  - Host: argmax-route tokens to memory entries, group tokens by entry into
    single-entry groups of <=16 slots, distribute groups evenly over 8 cores
    (G groups per core, S = 16*G token slots per core). Zero-pad unused slots.
  - Math rewrite (folding): instead of projecting each token's [256,1024]
    K/V slab through wk/wv, fold wk into the query side and wv into the
    context side:
        scores[t,h,m] = (qhat[t,h,:] . K_e[m,:]) / 8,  qhat = q_h @ wk_h^T
        (bk cancels in softmax)
        ctx[t,h,:]   = cbar[t,h,:] @ wv_h + bv,  cbar = attn @ V_e
    This cuts matmul FLOPs ~2x vs projecting slabs.
  - Device (per core, SPMD, no collectives): transposed-activation layout
    [features on partitions, token slots on free dim]. LN via ones-matmul
    partition reductions; per-token scalars broadcast via K=1 matmuls.
    All matmul inputs fp16 (weights cast host-side), fp32 accumulate,
    fp32 LN/softmax-denominator/residual arithmetic.
"""

import numpy as np
from contextlib import ExitStack

import concourse.bacc as bacc
import concourse.tile as tile
import concourse.mybir as mybir
from concourse.bass_utils import run_bass_kernel_spmd

B, L, NMEM, LMEM, D, H, DFF = 4, 256, 64, 256, 1024, 16, 4096
DK = D // H
BL = B * L
NCORES = 8
GSZ = 16                 # token slots per attention group (single entry each)
DC = D // 128            # 8 feature chunks
FC = DFF // 128          # 32 ff chunks
MT = LMEM // 128         # 2 memory-row chunks

f32 = mybir.dt.float32
f16 = mybir.dt.float16
AF = mybir.ActivationFunctionType
ALU = mybir.AluOpType

# svec column layout (per-partition scalar vectors, one [128] chunk per col)
SV_G0, SV_BE0, SV_BQ, SV_BV, SV_BO, SV_B2A, SV_B2B, SV_G1, SV_BE1 = (
    0, 8, 16, 24, 32, 40, 48, 56, 64)
SV_B1A, SV_B1B = 72, 104
SV_COLS = 136

TRACE = False            # test harness can flip this for a profiled run
LAST_RESULTS = None      # BassKernelResults of last run (for test harness)

_PROG_CACHE = {}


def _build(G):
    S = G * GSZ
    HS = H * S
    HG = H * GSZ          # free width of per-group score tiles (256)

    nc = bacc.Bacc("TRN2", target_bir_lowering=False, debug=False,
                   num_devices=NCORES)

    dt_in = lambda n, s, d: nc.dram_tensor(n, s, d, kind="ExternalInput").ap()
    decT = dt_in("decT", [D, S], f32)
    gdiff = dt_in("gdiff", [1, S], f32)
    svec_d = dt_in("svec", [128, SV_COLS], f32)
    wq_d = dt_in("wq16", [D, D], f16)
    wkT_d = dt_in("wkT16", [D, D], f16)
    wv_d = dt_in("wv16", [D, D], f16)
    wo_d = dt_in("wo16", [D, D], f16)
    # pre-tiled FF weights (host layout: see _tile_w1/_tile_w2)
    w1a_d = dt_in("w1a16", [DC * 128, DFF], f16)
    w2a_d = dt_in("w2a16", [DC * 128, DFF], f16)
    w1b_d = dt_in("w1b16", [DC * 128, DFF], f16)
    w2b_d = dt_in("w2b16", [DC * 128, DFF], f16)
    encT_d = dt_in("encT16", [G * D, LMEM], f16)
    vmem_d = dt_in("v16", [G * LMEM, D], f16)
    outT = nc.dram_tensor("outT", [D, S], f32, kind="ExternalOutput").ap()

    with tile.TileContext(nc) as tc, ExitStack() as ctx:
        P = lambda name, bufs, space=None: ctx.enter_context(
            tc.tile_pool(name=name, bufs=bufs, space=space)
            if space else tc.tile_pool(name=name, bufs=bufs))

        p_const = P("const", 1)
        p_tmp32 = P("tmp32", 2)
        p_tmp16 = P("tmp16", 2)
        p_x32 = P("x32", DC)
        p_x16 = P("x16", DC)
        p_c16 = P("c16", DC)
        p_st32 = P("st32", 2 * DC)
        p_st16 = P("st16", 2 * DC)
        p_h1 = P("h1", FC)
        p_out32 = P("out32", 4)
        p_bigw = P("bigw", 16)
        p_stat = P("stat", 5)
        p_ffw = P("ffw", 3)
        p_ps = P("ps", 6, space="PSUM")
        p_psbc = P("psbc", 2, space="PSUM")
        mid_pools = ExitStack()
        MP = lambda name, bufs: mid_pools.enter_context(
            tc.tile_pool(name=name, bufs=bufs))
        p_qhat = MP("qhat", DC)
        p_enc = MP("enc", 3)
        p_v = MP("v", 2 * MT)
        p_att = MP("att", 2 * G + 2)

        # ---- constants ----
        svec = p_const.tile([128, SV_COLS], f32)
        nc.sync.dma_start(out=svec[:], in_=svec_d[:, :])
        ones_c16 = p_const.tile([128, 1], f16)
        nc.vector.memset(ones_c16[:], 1.0)
        ones_r32 = p_const.tile([1, 128], f32)
        nc.vector.memset(ones_r32[:], 1.0)
        ones_r16 = p_const.tile([1, 128], f16)
        nc.vector.memset(ones_r16[:], 1.0)
        zcol = p_const.tile([128, 1], f32)
        nc.vector.memset(zcol[:], 0.0)
        eps1 = p_const.tile([1, 1], f32)
        nc.vector.memset(eps1[:], 1e-5)
        gd = p_const.tile([1, S], f32)
        nc.sync.dma_start(out=gd[:], in_=gdiff[:, :])

        def layernorm(src, g_col, b_col, dst_pool, dst16_pool,
                      tag32='x', tag16='x6', make16=True):
            """src: list of DC [128,S] f32 tiles -> (x32 list, x16 list)."""
            s16 = []
            for c in range(DC):
                t6 = p_tmp16.tile([128, S], f16, tag="ln16")
                nc.scalar.activation(t6[:], src[c][:], AF.Copy)
                s16.append(t6)
            ps_sum = p_ps.tile([1, S], f32, tag="ps")
            for c in range(DC):
                nc.tensor.matmul(ps_sum[:], lhsT=ones_c16[:], rhs=s16[c][:],
                                 start=(c == 0), stop=(c == DC - 1))
            ps_ssq = p_ps.tile([1, S], f32, tag="ps")
            for c in range(DC):
                sq = p_tmp16.tile([128, S], f16, tag="lnsq")
                nc.vector.tensor_mul(sq[:], s16[c][:], s16[c][:])
                nc.tensor.matmul(ps_ssq[:], lhsT=ones_c16[:], rhs=sq[:],
                                 start=(c == 0), stop=(c == DC - 1))
            mean = p_stat.tile([1, S], f32, tag="stat")
            nc.vector.tensor_scalar(mean[:], ps_sum[:], 1.0 / D, None, ALU.mult)
            msq = p_stat.tile([1, S], f32, tag="stat")
            nc.vector.tensor_mul(msq[:], mean[:], mean[:])
            var = p_stat.tile([1, S], f32, tag="stat")
            nc.vector.tensor_scalar(var[:], ps_ssq[:], 1.0 / D, None, ALU.mult)
            var2 = p_stat.tile([1, S], f32, tag="stat")
            nc.vector.tensor_sub(var2[:], var[:], msq[:])
            std = p_stat.tile([1, S], f32, tag="stat")
            nc.scalar.activation(std[:], var2[:], AF.Sqrt, bias=eps1[:])
            rstd = p_stat.tile([1, S], f32, tag="stat")
            nc.vector.reciprocal(rstd[:], std[:])
            ps_mb = p_psbc.tile([128, S], f32, tag="bc")
            nc.tensor.matmul(ps_mb[:], lhsT=ones_r32[:], rhs=mean[:],
                             start=True, stop=True)
            ps_rb = p_psbc.tile([128, S], f32, tag="bc")
            nc.tensor.matmul(ps_rb[:], lhsT=ones_r32[:], rhs=rstd[:],
                             start=True, stop=True)
            o32, o16 = [], []
            for c in range(DC):
                t = p_tmp32.tile([128, S], f32, tag="lnt")
                nc.vector.tensor_sub(t[:], src[c][:], ps_mb[:])
                t2 = p_tmp32.tile([128, S], f32, tag="lnt2")
                nc.vector.tensor_mul(t2[:], t[:], ps_rb[:])
                x = dst_pool.tile([128, S], f32, tag=tag32, name=f'{tag32}_{c}')
                nc.vector.tensor_scalar(x[:], t2[:], svec[:, g_col + c:g_col + c + 1],
                                        svec[:, b_col + c:b_col + c + 1],
                                        ALU.mult, ALU.add)
                if make16:
                    x6 = dst16_pool.tile([128, S], f16, tag=tag16,
                                         name=f'{tag16}_{c}')
                    nc.scalar.activation(x6[:], x[:], AF.Copy)
                    o16.append(x6)
                o32.append(x)
            return o32, o16

        # ---- stage A/B: load dec, LN0 ----
        early_pools = ExitStack()
        p_dec = early_pools.enter_context(tc.tile_pool(name="dec", bufs=DC))
        p_q16 = early_pools.enter_context(tc.tile_pool(name="q16", bufs=DC))
        dec = []
        for c in range(DC):
            t = p_dec.tile([128, S], f32)
            nc.sync.dma_start(out=t[:], in_=decT[c * 128:(c + 1) * 128, :])
            dec.append(t)
        x32, x16 = layernorm(dec, SV_G0, SV_BE0, p_x32, p_x16)

        # ---- stage C: q = x @ wq + bq  (transposed: [D', S]) ----
        def load_w(dram, nm):
            ts = []
            for c in range(DC):
                t = p_bigw.tile([128, D], f16, tag="bigw", name=f"{nm}{c}")
                nc.sync.dma_start(
                    out=t[:], in_=dram.rearrange("(c p) n -> p c n", p=128)[:, c, :])
                ts.append(t)
            return ts

        bw = load_w(wq_d, "wq")
        q16 = []
        for n in range(DC):
            ps = p_ps.tile([128, S], f32, tag="ps")
            for c in range(DC):
                nc.tensor.matmul(ps[:], lhsT=bw[c][:, n * 128:(n + 1) * 128],
                                 rhs=x16[c][:], start=(c == 0), stop=(c == DC - 1))
            qt = p_q16.tile([128, S], f16, tag="q")
            nc.vector.tensor_scalar(qt[:], ps[:], svec[:, SV_BQ + n:SV_BQ + n + 1],
                                    None, ALU.add)
            q16.append(qt)

        # ---- stage D: qhat[c][:, h*S:(h+1)*S] = wkT_h @ q_h ----
        bw2 = load_w(wkT_d, "wkT")
        qhat = [p_qhat.tile([128, HS], f16, tag='qhat', name=f'qhat{c}') for c in range(DC)]
        for h in range(H):
            rr = (h % 2) * 64
            for c in range(DC):
                ps = p_ps.tile([128, S], f32, tag="ps")
                nc.tensor.matmul(
                    ps[:],
                    lhsT=bw2[h // 2][rr:rr + 64, c * 128:(c + 1) * 128],
                    rhs=q16[h // 2][rr:rr + 64, :], start=True, stop=True)
                dst = qhat[c][:, h * S:(h + 1) * S]
                if (h * DC + c) % 2 == 0:
                    nc.vector.tensor_copy(dst, ps[:])
                else:
                    nc.scalar.activation(dst, ps[:], AF.Copy)

        early_pools.close()

        # ---- stage E: per-group attention ----
        cbar = qhat   # cbar reuses qhat storage: per-group columns of qhat
        # are dead after that group's score matmuls read them.
        qv = [qhat[c].rearrange("p (h t) -> p h t", h=H) for c in range(DC)]
        cbv = qv
        at_all = []
        for g in range(G):
            encg = p_enc.tile([128, DC * LMEM], f16)
            encv = encg.rearrange("p (c m) -> p c m", c=DC)
            nc.sync.dma_start(
                out=encv,
                in_=encT_d[g * D:(g + 1) * D, :].rearrange("(c p) m -> p c m", p=128))
            sl = slice(g * GSZ, (g + 1) * GSZ)
            # scores -> exp
            ex = []
            ps_sc = []
            for mc in range(MT):
                ps = p_ps.tile([128, HG], f32, tag="ps")
                for c in range(DC):
                    nc.tensor.matmul(ps[:], lhsT=encv[:, c, mc * 128:(mc + 1) * 128],
                                     rhs=qv[c][:, :, sl],
                                     start=(c == 0), stop=(c == DC - 1))
                ps_sc.append(ps)
            for mc in range(MT):
                e = p_att.tile([128, HG], f16, tag="att")
                nc.scalar.activation(e[:], ps_sc[mc][:], AF.Exp, bias=zcol[:],
                                     scale=0.125)
                ex.append(e)
            # denom -> reciprocal -> broadcast
            ps_den = p_ps.tile([1, HG], f32, tag="ps")
            for mc in range(MT):
                nc.tensor.matmul(ps_den[:], lhsT=ones_c16[:], rhs=ex[mc][:],
                                 start=(mc == 0), stop=(mc == MT - 1))
            den32 = p_stat.tile([1, HG], f32, tag="den", bufs=2)
            nc.vector.reciprocal(den32[:], ps_den[:])
            den16 = p_stat.tile([1, HG], f16, tag="den16", bufs=2)
            nc.vector.tensor_copy(den16[:], den32[:])
            ps_bc = p_psbc.tile([128, HG], f32, tag="bc")
            nc.tensor.matmul(ps_bc[:], lhsT=ones_r16[:], rhs=den16[:],
                             start=True, stop=True)
            at = []
            for mc in range(MT):
                a = p_att.tile([128, HG], f16, tag="att")
                nc.vector.tensor_mul(a[:], ex[mc][:], ps_bc[:])
                at.append(a)
            vg = []
            for mc in range(MT):
                vt = p_v.tile([128, D], f16)
                nc.sync.dma_start(
                    out=vt[:], in_=vmem_d[g * LMEM + mc * 128:g * LMEM + (mc + 1) * 128, :])
                vg.append(vt)
            at_all.append((at, vg))
        # cbar phase: after all score reads of qhat, overwrite qhat with cbar
        for g in range(G):
            at, vg = at_all[g]
            sl = slice(g * GSZ, (g + 1) * GSZ)
            for dtile in range(DC):
                ps = p_ps.tile([128, HG], f32, tag="ps")
                for mc in range(MT):
                    nc.tensor.matmul(ps[:], lhsT=vg[mc][:, dtile * 128:(dtile + 1) * 128],
                                     rhs=at[mc][:], start=(mc == 0), stop=(mc == MT - 1))
                if dtile % 2 == 0:
                    nc.vector.tensor_copy(
                        cbv[dtile][:, :, sl],
                        ps.rearrange("p (h t) -> p h t", h=H)[:, :, :])
                else:
                    nc.scalar.activation(
                        cbv[dtile][:, :, sl],
                        ps.rearrange("p (h t) -> p h t", h=H)[:, :, :], AF.Copy)

        # ---- stage F: ctx = cbar @ wv + bv  (per head) ----
        bw3 = load_w(wv_d, "wv")
        ctx16 = [p_c16.tile([128, S], f16, tag='ctx16', name=f'ctx16_{c}') for c in range(DC)]
        for h in range(H):
            rr = (h % 2) * 64
            ps = p_ps.tile([64, S], f32, tag="ps")
            for c in range(DC):
                nc.tensor.matmul(ps[:], lhsT=bw3[c][:, h * 64:(h + 1) * 64],
                                 rhs=cbar[c][:, h * S:(h + 1) * S],
                                 start=(c == 0), stop=(c == DC - 1))
            if h % 2 == 0:
                nc.vector.tensor_scalar(
                    ctx16[h // 2][rr:rr + 64, :], ps[:],
                    svec[rr:rr + 64, SV_BV + h // 2:SV_BV + h // 2 + 1], None,
                    ALU.add)
            else:
                nc.scalar.activation(
                    ctx16[h // 2][rr:rr + 64, :], ps[:], AF.Identity,
                    bias=svec[rr:rr + 64, SV_BV + h // 2:SV_BV + h // 2 + 1])

        mid_pools.close()
        p_ffw2 = ctx.enter_context(tc.tile_pool(name="ffw2", bufs=9))

        # ---- stage G: st = ctx @ wo + bo + x ----
        bw4 = load_w(wo_d, "wo")
        st32, st16 = [], []
        for n in range(DC):
            ps = p_ps.tile([128, S], f32, tag="ps")
            for c in range(DC):
                nc.tensor.matmul(ps[:], lhsT=bw4[c][:, n * 128:(n + 1) * 128],
                                 rhs=ctx16[c][:], start=(c == 0), stop=(c == DC - 1))
            s = p_st32.tile([128, S], f32, tag="stm", name=f"st_{n}")
            nc.vector.scalar_tensor_tensor(s[:], ps[:],
                                           svec[:, SV_BO + n:SV_BO + n + 1],
                                           x32[n][:], ALU.add, ALU.add)
            s6 = p_st16.tile([128, S], f16, tag="s16", name=f"st16_{n}")
            nc.scalar.activation(s6[:], s[:], AF.Copy)
            st32.append(s)
            st16.append(s6)

        dmae = [nc.sync, nc.scalar, nc.gpsimd]

        def ffn(in16, res32, w1d, w2d, b1_col, b2_col, evac, pw1, pw2):
            """positionwise FF: evac(n, psum_final, res32[n])."""
            hts = []
            for fb in range(8):
                wt = pw1.tile([128, DC * 512], f16, tag="ffw")
                wtv = wt.rearrange("p (c n) -> p c n", c=DC)
                dmae[fb % 3].dma_start(
                    out=wt[:], in_=w1d[fb * 128:(fb + 1) * 128, :])
                for j in range(4):
                    f = fb * 4 + j
                    ps = p_ps.tile([128, S], f32, tag="ps")
                    for c in range(DC):
                        nc.tensor.matmul(ps[:], lhsT=wtv[:, c, j * 128:(j + 1) * 128],
                                         rhs=in16[c][:], start=(c == 0), stop=(c == DC - 1))
                    ht = p_h1.tile([128, S], f16, tag="h1")
                    nc.scalar.activation(ht[:], ps[:], AF.Relu,
                                         bias=svec[:, b1_col + f:b1_col + f + 1])
                    hts.append(ht)
            outs = []
            for n in range(DC):
                wt = pw2.tile([128, FC * 128], f16, tag="ffw")
                wtv = wt.rearrange("p (c n) -> p c n", c=FC)
                hw2 = FC * 128 // 2
                dmae[n % 3].dma_start(
                    out=wt[:, 0:hw2], in_=w2d[n * 128:(n + 1) * 128, 0:hw2])
                dmae[(n + 1) % 3].dma_start(
                    out=wt[:, hw2:], in_=w2d[n * 128:(n + 1) * 128, hw2:])
                ps = p_ps.tile([128, S], f32, tag="ps")
                for fc in range(FC):
                    nc.tensor.matmul(ps[:], lhsT=wtv[:, fc, :], rhs=hts[fc][:],
                                     start=(fc == 0), stop=(fc == FC - 1))
                outs.append(evac(n, ps, b2_col))
            return outs

        # ---- stage H: FFa ----
        def evac_ffa(n, ps, b2_col):
            s = p_st32.tile([128, S], f32, tag="stm", name=f"st2_{n}")
            nc.vector.scalar_tensor_tensor(s[:], ps[:],
                                           svec[:, b2_col + n:b2_col + n + 1],
                                           st32[n][:], ALU.add, ALU.add)
            return s
        st2 = ffn(st16, st32, w1a_d, w2a_d, SV_B1A, SV_B2A, evac_ffa,
                  p_ffw, p_ffw2)

        # ---- stage I: LN1, gate, y ----
        stn32, _ = layernorm(st2, SV_G1, SV_BE1, p_st32, p_st16,
                             tag32='stm', make16=False)
        sig = p_stat.tile([1, S], f32, tag="sig", bufs=1)
        nc.scalar.activation(sig[:], gd[:], AF.Sigmoid, bias=zcol[0:1, :])
        ps_gb = p_psbc.tile([128, S], f32, tag="bc")
        nc.tensor.matmul(ps_gb[:], lhsT=ones_r32[:], rhs=sig[:], start=True, stop=True)
        y32, y16 = [], []
        for c in range(DC):
            t = p_tmp32.tile([128, S], f32, tag="yt")
            nc.vector.tensor_mul(t[:], stn32[c][:], ps_gb[:])
            y = p_st32.tile([128, S], f32, tag="stm", name=f"y_{c}")
            nc.vector.tensor_add(y[:], t[:], x32[c][:])
            y6 = p_st16.tile([128, S], f16, tag="s16", name=f"y16_{c}")
            nc.scalar.activation(y6[:], y[:], AF.Copy)
            y32.append(y)
            y16.append(y6)

        # ---- stage J: FFb -> out ----
        def evac_ffb(n, ps, b2_col):
            o = p_out32.tile([128, S], f32)
            nc.vector.scalar_tensor_tensor(o[:], ps[:],
                                           svec[:, b2_col + n:b2_col + n + 1],
                                           y32[n][:], ALU.add, ALU.add)
            nc.sync.dma_start(out=outT[n * 128:(n + 1) * 128, :], in_=o[:])
            return o
        ffn(y16, y32, w1b_d, w2b_d, SV_B1B, SV_B2B, evac_ffb,
            p_ffw2, p_ffw)

    nc.compile()
    return nc


def _chunk_cols(vec, n):
    """[n*128] -> [128, n] (column c = chunk c)."""
    return np.ascontiguousarray(vec.reshape(n, 128).T)


def kernel(**inputs):
    global LAST_RESULTS
    gi = lambda n: np.asarray(inputs[n])
    dec = gi("dec_output").astype(np.float32).reshape(BL, D)
    gl = gi("gate_logits").astype(np.float32).reshape(BL, 2)
    ma = gi("mem_attn").astype(np.float32).reshape(BL, NMEM)
    enc = gi("enc_out_mem").astype(np.float32)
    tgt = gi("tgt_emb_mem").astype(np.float32)

    samples = ma.argmax(-1)
    groups = []
    for e in range(NMEM):
        toks = np.nonzero(samples == e)[0]
        for i in range(0, len(toks), GSZ):
            groups.append((e, toks[i:i + GSZ]))
    G = (len(groups) + NCORES - 1) // NCORES
    while len(groups) < G * NCORES:
        groups.append((0, np.empty([0], np.int64)))
    S = G * GSZ

    # fp16 weights (shared across cores)
    wq16 = gi("wq").astype(np.float16)
    wkT16 = np.ascontiguousarray(gi("wk").astype(np.float32).T).astype(np.float16)
    wv16 = gi("wv").astype(np.float16)
    wo16 = gi("wo").astype(np.float16)
    def _tile_w1(w):  # [D, DFF] -> [fb*128+p, (c, fi)] contiguous blocks
        return np.ascontiguousarray(
            w.reshape(DC, 128, 8, 512).transpose(2, 1, 0, 3).reshape(DC * 128, DFF))

    def _tile_w2(w):  # [DFF, D] -> [nb*128+p, (fc, n)] contiguous blocks
        return np.ascontiguousarray(
            w.reshape(FC, 128, DC, 128).transpose(2, 1, 0, 3).reshape(DC * 128, DFF))

    w1a16 = _tile_w1(gi("w1a").astype(np.float16))
    w2a16 = _tile_w2(gi("w2a").astype(np.float16))
    w1b16 = _tile_w1(gi("w1b").astype(np.float16))
    w2b16 = _tile_w2(gi("w2b").astype(np.float16))
    svec = np.zeros([128, SV_COLS], np.float32)
    svec[:, SV_G0:SV_G0 + 8] = _chunk_cols(gi("g0").astype(np.float32), 8)
    svec[:, SV_BE0:SV_BE0 + 8] = _chunk_cols(gi("be0").astype(np.float32), 8)
    svec[:, SV_BQ:SV_BQ + 8] = _chunk_cols(gi("bq").astype(np.float32), 8)
    svec[:, SV_BV:SV_BV + 8] = _chunk_cols(gi("bv").astype(np.float32), 8)
    svec[:, SV_BO:SV_BO + 8] = _chunk_cols(gi("bo").astype(np.float32), 8)
    svec[:, SV_B2A:SV_B2A + 8] = _chunk_cols(gi("b2a").astype(np.float32), 8)
    svec[:, SV_B2B:SV_B2B + 8] = _chunk_cols(gi("b2b").astype(np.float32), 8)
    svec[:, SV_G1:SV_G1 + 8] = _chunk_cols(gi("g1").astype(np.float32), 8)
    svec[:, SV_BE1:SV_BE1 + 8] = _chunk_cols(gi("be1").astype(np.float32), 8)
    svec[:, SV_B1A:SV_B1A + 32] = _chunk_cols(gi("b1a").astype(np.float32), 32)
    svec[:, SV_B1B:SV_B1B + 32] = _chunk_cols(gi("b1b").astype(np.float32), 32)

    encT16 = np.ascontiguousarray(enc.transpose(0, 2, 1)).astype(np.float16)  # [NMEM, D, LMEM]
    tgt16 = tgt.astype(np.float16)                                            # [NMEM, LMEM, D]
    gdiff_all = gl[:, 1] - gl[:, 0]

    in_maps = []
    core_slots = []   # per core: (token_idx array, slot array)
    for k in range(NCORES):
        cg = groups[k * G:(k + 1) * G]
        decT = np.zeros([D, S], np.float32)
        gdif = np.zeros([1, S], np.float32)
        encTc = np.empty([G * D, LMEM], np.float16)
        vc = np.empty([G * LMEM, D], np.float16)
        tok_idx, slot_idx = [], []
        for g, (e, toks) in enumerate(cg):
            encTc[g * D:(g + 1) * D] = encT16[e]
            vc[g * LMEM:(g + 1) * LMEM] = tgt16[e]
            if len(toks):
                sl = g * GSZ + np.arange(len(toks))
                decT[:, sl] = dec[toks].T
                gdif[0, sl] = gdiff_all[toks]
                tok_idx.append(toks)
                slot_idx.append(sl)
        core_slots.append((
            np.concatenate(tok_idx) if tok_idx else np.empty([0], np.int64),
            np.concatenate(slot_idx) if slot_idx else np.empty([0], np.int64)))
        in_maps.append({
            "decT": decT, "gdiff": gdif, "svec": svec,
            "wq16": wq16, "wkT16": wkT16, "wv16": wv16, "wo16": wo16,
            "w1a16": w1a16, "w2a16": w2a16, "w1b16": w1b16, "w2b16": w2b16,
            "encT16": encTc, "v16": vc,
        })

    if G not in _PROG_CACHE:
        _PROG_CACHE[G] = _build(G)
    nc = _PROG_CACHE[G]

    kwargs = {}
    if TRACE:
        kwargs = dict(trace=True, trace_cores=list(range(NCORES)))
    res = run_bass_kernel_spmd(nc, in_maps, core_ids=list(range(NCORES)), **kwargs)
    LAST_RESULTS = res

    out = np.empty([BL, D], np.float32)
    for k in range(NCORES):
        toks, slots = core_slots[k]
        if len(toks):
            out[toks] = res.results[k]["outT"][:, slots].T
    return out.reshape(B, L, D)

